# revision 2
# baseline (speedup 1.0000x reference)
"""Multi-head attention (B=4, T=2048, D=1024, H=16) on 8 NeuronCores.

Sharding: core c handles batch b=c//2 and head-group hg=c%2 (8 heads).
Per core: qk^T projection (transposed layout), v projection (natural),
scores-transposed attention with exp (no max subtraction; |scores| < ~2),
rowsum via a ones-column appended to v, out-projection from o^T.
Host sums the two tensor-parallel partials per batch and adds b_out.

v3: single fused pipeline. The scalar engine (exp) is the bottleneck
during attention, so all other work hides in its shadow:
  A_v -> A_qk(0) -> [B(0) || A_qk(1)] -> ... -> [B(3) || C(mc-1)] -> C(3)
One shared [128,512] psum pool (2 banks) serves the v/qk/out projection
groups; scores use 4 banks (2-head batched exp, N=1024 ACTIVATE), AV
accumulators 2 banks. Bias-adds and psum moves ride the vector engine.
"""
import sys

sys.path.insert(0, "/opt/trn_rl_repo")

import numpy as np
import ml_dtypes

T = 2048
D = 1024
NH = 16
DH = 64
TK = T // 128          # 16 t-tiles
KT = D // 128          # 8 contraction tiles
HL = NH // 2           # 8 heads per core
MC = 512               # m-chunk width
NMC = T // MC          # 4 chunks
SCALE = 1.0 / np.sqrt(DH)

_PROGRAM = None


def _build_program():
    import concourse.bacc as bacc
    import concourse.tile as tile
    from concourse import mybir

    f32 = mybir.dt.float32
    bf16 = mybir.dt.bfloat16
    f8 = mybir.dt.float8e4
    EXP = mybir.ActivationFunctionType.Exp

    nc = bacc.Bacc("TRN2", target_bir_lowering=False)

    xT = nc.dram_tensor("xT", [D, T], bf16, kind="ExternalInput")
    wqk = nc.dram_tensor("wqk", [D, D], bf16, kind="ExternalInput")
    wv = nc.dram_tensor("wv", [D, 512], bf16, kind="ExternalInput")
    wout = nc.dram_tensor("wout", [512, D], bf16, kind="ExternalInput")
    bqk = nc.dram_tensor("bqk", [D, 1], f32, kind="ExternalInput")
    bvb = nc.dram_tensor("bvb", [1, 512], f32, kind="ExternalInput")
    out_p = nc.dram_tensor("out_p", [T, D], f32, kind="ExternalOutput")
    # scratch for rowsum-reciprocal partition broadcast (SBUF sources cannot
    # have zero partition step in DMA; DRAM sources can)
    rscr = nc.dram_tensor("rscr", [4, NMC, 2, MC], f32)
    rscr2 = nc.dram_tensor("rscr2", [4, NMC, 2, MC], f32)

    with tile.TileContext(nc) as tc:
        with (
            tc.tile_pool(name="persist", bufs=1) as persist,
            tc.tile_pool(name="xtp", bufs=1) as xtp,
            tc.tile_pool(name="wp", bufs=1) as wp,
            tc.tile_pool(name="ptp", bufs=10) as ptp,
            tc.tile_pool(name="osb", bufs=2) as osb,
            tc.tile_pool(name="rbp", bufs=2) as rbp,
            tc.tile_pool(name="tmpb", bufs=1) as tmpb,
            tc.tile_pool(name="cop", bufs=2) as cop,
            tc.tile_pool(name="gp", bufs=2, space="PSUM") as gp,
            tc.tile_pool(name="sps", bufs=2, space="PSUM") as sps,
            tc.tile_pool(name="ops", bufs=1, space="PSUM") as ops,
        ):
            # persistent SBUF tensors
            qkT = [persist.tile([128, T], bf16, tag=f"qk{j}", name=f"qk{j}")
                   for j in range(KT)]
            vsb = [persist.tile([128, HL, DH + 1], bf16, tag=f"v{t}",
                                name=f"v{t}") for t in range(TK)]
            oT = [persist.tile([128, T], bf16, tag=f"ot{p}", name=f"ot{p}")
                  for p in range(4)]
            bv_bc = persist.tile([128, 512], f32, tag="bvbc", name="bvbc")
            wrm = persist.tile([1, 1], f32, tag="wrm", name="wrm")
            xt_all = xtp.tile([128, KT, T], bf16, tag="xt", name="xt")
            xT_sb = [xt_all[:, k, :] for k in range(KT)]
            wqk_all = wp.tile([128, KT, D], bf16, tag="wq", name="wq")
            wqk_sb = [wqk_all[:, k, :] for k in range(KT)]
            wout_all = wp.tile([128, 4, D], bf16, tag="wo", name="wo")
            wout_sb = [wout_all[:, j, :] for j in range(4)]
            bqk_all = persist.tile([128, KT], f32, tag="bqa", name="bqa")

            # warm the exp activation table during the initial DMA wait
            nc.vector.memset(wrm[:], 0.0)
            nc.scalar.activation(wrm[:], wrm[:], EXP)

            nc.sync.dma_start(out=bv_bc[:], in_=bvb[0:1, :].to_broadcast([128, 512]))
            nc.sync.dma_start(
                out=bqk_all[:],
                in_=bqk[:, 0].rearrange("(k p) -> p k", p=128))
            # ---- A_v: v projection (natural layout), bias, ones column ----
            if True:
                wv_all = wp.tile([128, KT, 512], bf16, tag="wv", name="wv")
                wv_sb = [wv_all[:, k, :] for k in range(KT)]
                # input DMA: v weights, first half of xT columns, qk weights,
                # second half, out weights (needed last)
                nc.sync.dma_start(
                    out=wv_all[:],
                    in_=wv[:, :].rearrange("(k p) n -> p k n", p=128))
                for half in range(2):
                    nc.sync.dma_start(
                        out=xt_all[:, :, 1024 * half:1024 * (half + 1)],
                        in_=xT[:, 1024 * half:1024 * (half + 1)].rearrange(
                            "(k p) n -> p k n", p=128))
                    if half == 0:
                        nc.sync.dma_start(
                            out=wqk_all[:],
                            in_=wqk[:, :].rearrange("(k p) n -> p k n", p=128))
                nc.sync.dma_start(
                    out=wout_all[:],
                    in_=wout[:, :].rearrange("(j p) n -> p j n", p=128))

                def emit_v(ts):
                    for t in ts:
                        ps = gp.tile([128, 512], f32, tag="gp", name="gp")
                        for k in range(KT):
                            nc.tensor.matmul(
                                ps[:], xT_sb[k][:, 128 * t:128 * (t + 1)],
                                wv_sb[k][:],
                                start=(k == 0), stop=(k == KT - 1))
                        nc.vector.tensor_add(
                            vsb[t][:, :, 0:DH],
                            ps[:].rearrange("p (h d) -> p h d", h=HL),
                            bv_bc[:].rearrange("p (h d) -> p h d", h=HL))
                        nc.vector.memset(vsb[t][:, :, DH:DH + 1], 1.0)

                # first half of A_v needs only wv + xT cols 0-1023; the rest
                # hides in B(0)'s scalar-bound shadow (emitted in the B loop)
                emit_v(range(TK))

            # ---- A_qk(hp): q,k transposed-layout projection for one pair ----
            def emit_qk(hp):
                for j in (4 + hp, hp):
                    for c in range(NMC):
                        ps = gp.tile([128, MC], f32, tag="gp", name="gp")
                        for k in range(KT):
                            nc.tensor.matmul(
                                ps[:],
                                wqk_sb[k][:, 128 * j:128 * (j + 1)],
                                xT_sb[k][:, MC * c:MC * (c + 1)],
                                start=(k == 0), stop=(k == KT - 1))
                        nc.vector.tensor_scalar_add(
                            qkT[j][:, MC * c:MC * (c + 1)], ps[:],
                            bqk_all[:, j:j + 1])

            # ---- C chunk: out-projection for one 512-query range ----
            def emit_c(mc):
                for t in range(4 * mc, 4 * (mc + 1)):
                    for ch in range(2):
                        ps = gp.tile([128, 512], f32, tag="gp", name="gp")
                        for j in range(4):
                            nc.tensor.matmul(
                                ps[:],
                                oT[j][:, 128 * t:128 * (t + 1)],
                                wout_sb[j][:, 512 * ch:512 * (ch + 1)],
                                start=(j == 0), stop=(j == 3))
                        ot = cop.tile([128, 512], f32, tag="co", name="co")
                        nc.vector.tensor_copy(ot[:], ps[:])
                        nc.sync.dma_start(
                            out=out_p[128 * t:128 * (t + 1),
                                      512 * ch:512 * (ch + 1)],
                            in_=ot[:])

            emit_qk(0)

            # ---- B(hp): attention for head pair hp, qk(hp+1)/C in shadow ----
            for hp in range(4):
                qt = qkT[hp]
                kt = qkT[4 + hp]
                hA, hB = 2 * hp, 2 * hp + 1
                for mc in range(NMC):
                    ptg = []
                    for nt in range(TK):
                        if nt % 2 == 0:
                            ptg.append(ptp.tile([128, 2, 2, MC], bf16,
                                                tag="pt", name="pt"))
                        pt = ptg[nt // 2]
                        s = sps.tile([128, 2, MC], f32, tag="s", name="s")
                        nc.tensor.matmul(
                            s[:, 0, :], kt[0:64, 128 * nt:128 * (nt + 1)],
                            qt[0:64, MC * mc:MC * (mc + 1)],
                            start=True, stop=True, tile_position=(0, 0))
                        nc.tensor.matmul(
                            s[:, 1, :], kt[64:128, 128 * nt:128 * (nt + 1)],
                            qt[64:128, MC * mc:MC * (mc + 1)],
                            start=True, stop=True, tile_position=(64, 0))
                        nc.scalar.activation(pt[:, nt % 2, :, :], s[:, :, :],
                                             EXP, scale=float(SCALE))
                    oA = ops.tile([65, MC], f32, tag="oA", name="oA")
                    oB = ops.tile([65, MC], f32, tag="oB", name="oB")
                    for nt in range(TK):
                        nc.tensor.matmul(oA[:], vsb[nt][:, hA, :],
                                         ptg[nt // 2][:, nt % 2, 0, :],
                                         start=(nt == 0), stop=(nt == TK - 1))
                        nc.tensor.matmul(oB[:], vsb[nt][:, hB, :],
                                         ptg[nt // 2][:, nt % 2, 1, :],
                                         start=(nt == 0), stop=(nt == TK - 1))
                    # move the accumulators to SBUF immediately so the psum
                    # banks free for the next chunk; tail runs out of SBUF
                    sA = osb.tile([65, MC], f32, tag="sA", name="sA")
                    sB = osb.tile([65, MC], f32, tag="sB", name="sB")
                    nc.vector.tensor_copy(sA[:], oA[:])
                    nc.vector.tensor_copy(sB[:], oB[:])
                    # rowsums sit in partition 64; recip in-lane then
                    # DMA-broadcast to partitions 0-63
                    rA = rbp.tile([128, MC], f32, tag="rA", name="rA")
                    rB = rbp.tile([128, MC], f32, tag="rB", name="rB")
                    nc.vector.reciprocal(rA[64:65, :], sA[64:65, :])
                    nc.vector.reciprocal(rB[64:65, :], sB[64:65, :])
                    nc.sync.dma_start(out=rscr[hp, mc, 0, :], in_=rA[64:65, :])
                    nc.sync.dma_start(out=rscr[hp, mc, 1, :], in_=rB[64:65, :])
                    nc.sync.dma_start(
                        out=rA[0:64, :],
                        in_=rscr[hp, mc, 0:1, :].to_broadcast([64, MC]))
                    nc.sync.dma_start(
                        out=rB[0:64, :],
                        in_=rscr[hp, mc, 1:2, :].to_broadcast([64, MC]))
                    # normalize: head A direct; head B via tmp + DMA shift
                    nc.vector.tensor_mul(
                        oT[hp][0:64, MC * mc:MC * (mc + 1)],
                        sA[0:64, :], rA[0:64, :])
                    tB = tmpb.tile([64, MC], bf16, tag="tB", name="tB")
                    nc.vector.tensor_mul(tB[:], sB[0:64, :], rB[0:64, :])
                    nc.sync.dma_start(
                        out=oT[hp][64:128, MC * mc:MC * (mc + 1)],
                        in_=tB[:])
                if hp < 3:
                    emit_qk(hp + 1)
            # C emitted last => lowest scheduler priority => pure gap-filler:
            # chunks run in B(3)'s PE slack as their oT inputs complete,
            # never ahead of same-engine attention work
            for mc in range(NMC):
                emit_c(mc)

    nc.compile()
    return nc


def _get_program():
    global _PROGRAM
    if _PROGRAM is None:
        _PROGRAM = _build_program()
    return _PROGRAM


def _bf16(x):
    return np.ascontiguousarray(x, dtype=ml_dtypes.bfloat16)


def _make_in_maps(x, w_qkv, b_qkv, w_out):
    in_maps = []
    for c in range(8):
        b, hg = c // 2, c % 2
        qs, ks, vs = 512 * hg, D + 512 * hg, 2 * D + 512 * hg
        in_maps.append({
            "xT": _bf16(x[b].T),
            "wqk": _bf16(np.concatenate(
                [w_qkv[:, qs:qs + 512], w_qkv[:, ks:ks + 512]], axis=1)),
            "wv": _bf16(w_qkv[:, vs:vs + 512]),
            "wout": _bf16(w_out[512 * hg:512 * hg + 512, :]),
            "bqk": np.ascontiguousarray(np.concatenate(
                [b_qkv[qs:qs + 512], b_qkv[ks:ks + 512]])[:, None],
                dtype=np.float32),
            "bvb": np.ascontiguousarray(b_qkv[vs:vs + 512][None, :],
                                        dtype=np.float32),
        })
    return in_maps


def kernel(x, w_qkv, b_qkv, w_out, b_out, _trace=False):
    from concourse.bass_utils import run_bass_kernel_spmd

    x = np.asarray(x, dtype=np.float32)
    w_qkv = np.asarray(w_qkv, dtype=np.float32)
    b_qkv = np.asarray(b_qkv, dtype=np.float32)
    w_out = np.asarray(w_out, dtype=np.float32)
    b_out = np.asarray(b_out, dtype=np.float32)

    nc = _get_program()
    in_maps = _make_in_maps(x, w_qkv, b_qkv, w_out)
    kres = run_bass_kernel_spmd(nc, in_maps, list(range(8)), trace=_trace)
    res = kres.results

    B = x.shape[0]
    out = np.empty((B, T, D), dtype=np.float32)
    for b in range(B):
        out[b] = res[2 * b]["out_p"] + res[2 * b + 1]["out_p"] + b_out
    if _trace:
        return out, kres
    return out
